# revision 6
# baseline (speedup 1.0000x reference)
"""CrossEntropyLoss kernel for Trainium2 (8 NeuronCores, data-parallel over rows).

Math: reference computes mean_n( sum_c target[n,c] * (-log_softmax(pred)[n,c]) ).
Decompose per row n:
    ce[n] = tsum[n] * lse[n] - dot[n]
where
    tsum[n] = sum_c target[n,c]
    lse[n]  = log(sum_c exp(pred[n,c]))    (pred ~ N(0,1) -> no overflow, max-sub
                                            subtraction is unnecessary in fp32)
    dot[n]  = sum_c pred[n,c]*target[n,c]
Engine split per [128, F] tile (rows on partitions, C chunked along free dim):
    ACT : exp + free-dim accumulate   -> sumexp partials
    DVE : tensor_tensor_reduce (mult,add) -> dot partials
    DVE : tensor_reduce (add)             -> tsum partials
All three stream each tile once; DMA of pred+target is the roofline bound.
Each core reduces its 1024 rows to a [128, 1] partial CE sum; host sums the
8 x 128 partials and divides by N.
"""

import numpy as np

import concourse.bacc as bacc
import concourse.tile as tile
from concourse import mybir
from concourse import bass_utils

N = 8192
C = 32000
N_CORES = 8
ROWS = N // N_CORES          # rows per core
P = 128                      # SBUF partitions
F = 2000                     # columns per chunk
OUT_NAME = "ce_part"

_f32 = mybir.dt.float32
_AF = mybir.ActivationFunctionType
_ALU = mybir.AluOpType
_AX = mybir.AxisListType


def build_program(rows=ROWS, cols=C, chunk=F):
    """Build the per-core Bass program (SPMD: same program on all cores)."""
    rb_count = rows // P
    nch = cols // chunk
    assert rows % P == 0 and cols % chunk == 0

    nc = bacc.Bacc(None, target_bir_lowering=False)
    pred = nc.dram_tensor("prediction", [rows, cols], _f32, kind="ExternalInput")
    tgt = nc.dram_tensor("target", [rows, cols], _f32, kind="ExternalInput")
    out = nc.dram_tensor(OUT_NAME, [P, 1], _f32, kind="ExternalOutput")

    with tile.TileContext(nc) as tc:
        with (
            tc.tile_pool(name="io", bufs=4) as io,
            tc.tile_pool(name="scr", bufs=2) as scr,
            tc.tile_pool(name="acc", bufs=2) as accp,
            tc.tile_pool(name="fin", bufs=1) as fin,
        ):
            se_all = fin.tile([P, rb_count], _f32)   # sumexp per row-block
            pr_all = fin.tile([P, rb_count], _f32)   # dot(pred,target) per row-block
            ts_all = fin.tile([P, rb_count], _f32)   # sum(target) per row-block

            for rb in range(rb_count):
                se_cols = accp.tile([P, nch], _f32)
                pr_cols = accp.tile([P, nch], _f32)
                ts_cols = accp.tile([P, nch], _f32)
                r0 = rb * P
                for j in range(nch):
                    c0 = j * chunk
                    p_t = io.tile([P, chunk], _f32)
                    t_t = io.tile([P, chunk], _f32)
                    nc.sync.dma_start(p_t[:, :], pred[r0 : r0 + P, c0 : c0 + chunk])
                    nc.sync.dma_start(t_t[:, :], tgt[r0 : r0 + P, c0 : c0 + chunk])
                    e_scr = scr.tile([P, chunk], _f32)
                    nc.scalar.activation(
                        e_scr[:, :], p_t[:, :], _AF.Exp,
                        accum_out=se_cols[:, j : j + 1],
                    )
                    pr_scr = scr.tile([P, chunk], _f32)
                    nc.vector.scalar_tensor_tensor(
                        pr_scr[:, :],
                        p_t[:, :],
                        1.0,
                        t_t[:, :],
                        _ALU.mult,
                        _ALU.mult,
                        accum_out=pr_cols[:, j : j + 1],
                    )
                    nc.vector.tensor_reduce(
                        ts_cols[:, j : j + 1], t_t[:, :], axis=_AX.X, op=_ALU.add
                    )
                nc.vector.tensor_reduce(
                    se_all[:, rb : rb + 1], se_cols[:, :], axis=_AX.X, op=_ALU.add
                )
                nc.vector.tensor_reduce(
                    pr_all[:, rb : rb + 1], pr_cols[:, :], axis=_AX.X, op=_ALU.add
                )
                nc.vector.tensor_reduce(
                    ts_all[:, rb : rb + 1], ts_cols[:, :], axis=_AX.X, op=_ALU.add
                )

            lse = fin.tile([P, rb_count], _f32)
            nc.scalar.activation(lse[:, :], se_all[:, :], _AF.Ln)
            tl = fin.tile([P, rb_count], _f32)
            nc.vector.tensor_mul(tl[:, :], ts_all[:, :], lse[:, :])
            ce_rb = fin.tile([P, rb_count], _f32)
            nc.vector.tensor_sub(ce_rb[:, :], tl[:, :], pr_all[:, :])
            ce1 = fin.tile([P, 1], _f32)
            nc.vector.tensor_reduce(ce1[:, :], ce_rb[:, :], axis=_AX.X, op=_ALU.add)
            nc.sync.dma_start(out[:, :], ce1[:, :])

    nc.finalize()
    return nc


_cached_nc = None


def _get_program():
    global _cached_nc
    if _cached_nc is None:
        _cached_nc = build_program()
    return _cached_nc


def kernel(prediction: np.ndarray, target: np.ndarray) -> np.ndarray:
    prediction = np.ascontiguousarray(prediction, dtype=np.float32)
    target = np.ascontiguousarray(target, dtype=np.float32)
    assert prediction.shape == (N, C) and target.shape == (N, C)

    nc = _get_program()
    in_maps = [
        {
            "prediction": prediction[i * ROWS : (i + 1) * ROWS],
            "target": target[i * ROWS : (i + 1) * ROWS],
        }
        for i in range(N_CORES)
    ]
    res = bass_utils.run_bass_kernel_spmd(nc, in_maps, core_ids=list(range(N_CORES)))
    total = 0.0
    for r in res.results:
        total += float(r[OUT_NAME].astype(np.float64).sum())
    return np.asarray(total / N, dtype=np.float32)


# revision 10
# speedup vs baseline: 88.1686x; 88.1686x over previous
"""CrossEntropyLoss kernel for Trainium2 (8 NeuronCores, data-parallel over rows).

Math: reference computes mean_n( sum_c target[n,c] * (-log_softmax(pred)[n,c]) ).
Decompose per row n:
    ce[n] = tsum[n] * lse[n] - dot[n]
where
    tsum[n] = sum_c target[n,c]
    lse[n]  = log(sum_c exp(pred[n,c]))    (pred ~ N(0,1) -> no overflow, max-sub
                                            subtraction is unnecessary in fp32)
    dot[n]  = sum_c pred[n,c]*target[n,c]
Engine split per [128, F] tile (rows on partitions, C chunked along free dim):
    ACT : exp + free-dim accumulate   -> sumexp partials
    DVE : tensor_tensor_reduce (mult,add) -> dot partials
    DVE : tensor_reduce (add)             -> tsum partials
All three stream each tile once; DMA of pred+target is the roofline bound.
Each core reduces its 1024 rows to a [128, 1] partial CE sum; host sums the
8 x 128 partials and divides by N.
"""

import numpy as np

import concourse.bacc as bacc
import concourse.tile as tile
from concourse import mybir

N = 8192
C = 32000
N_CORES = 8
ROWS = N // N_CORES          # rows per core
P = 128                      # SBUF partitions
F = 2000                     # columns per chunk
OUT_NAME = "ce_part"

_f32 = mybir.dt.float32
_AF = mybir.ActivationFunctionType
_ALU = mybir.AluOpType
_AX = mybir.AxisListType


def build_program(rows=ROWS, cols=C, chunk=F):
    """Build the per-core Bass program (SPMD: same program on all cores)."""
    rb_count = rows // P
    nch = cols // chunk
    assert rows % P == 0 and cols % chunk == 0

    nc = bacc.Bacc(None, target_bir_lowering=False)
    pred = nc.dram_tensor("prediction", [rows, cols], _f32, kind="ExternalInput")
    tgt = nc.dram_tensor("target", [rows, cols], _f32, kind="ExternalInput")
    out = nc.dram_tensor(OUT_NAME, [P, 1], _f32, kind="ExternalOutput")

    with tile.TileContext(nc) as tc:
        with (
            tc.tile_pool(name="io", bufs=4) as io,
            tc.tile_pool(name="scr", bufs=2) as scr,
            tc.tile_pool(name="acc", bufs=2) as accp,
            tc.tile_pool(name="fin", bufs=1) as fin,
        ):
            se_all = fin.tile([P, rb_count], _f32)   # sumexp per row-block
            pr_all = fin.tile([P, rb_count], _f32)   # dot(pred,target) per row-block
            ts_all = fin.tile([P, rb_count], _f32)   # sum(target) per row-block

            for rb in range(rb_count):
                se_cols = accp.tile([P, nch], _f32)
                pr_cols = accp.tile([P, nch], _f32)
                ts_cols = accp.tile([P, nch], _f32)
                r0 = rb * P
                for j in range(nch):
                    c0 = j * chunk
                    p_t = io.tile([P, chunk], _f32)
                    t_t = io.tile([P, chunk], _f32)
                    nc.sync.dma_start(p_t[:, :], pred[r0 : r0 + P, c0 : c0 + chunk])
                    nc.sync.dma_start(t_t[:, :], tgt[r0 : r0 + P, c0 : c0 + chunk])
                    e_scr = scr.tile([P, chunk], _f32)
                    nc.scalar.activation(
                        e_scr[:, :], p_t[:, :], _AF.Exp,
                        accum_out=se_cols[:, j : j + 1],
                    )
                    pr_scr = scr.tile([P, chunk], _f32)
                    nc.vector.scalar_tensor_tensor(
                        pr_scr[:, :],
                        p_t[:, :],
                        1.0,
                        t_t[:, :],
                        _ALU.mult,
                        _ALU.mult,
                        accum_out=pr_cols[:, j : j + 1],
                    )
                    nc.vector.tensor_reduce(
                        ts_cols[:, j : j + 1], t_t[:, :], axis=_AX.X, op=_ALU.add
                    )
                nc.vector.tensor_reduce(
                    se_all[:, rb : rb + 1], se_cols[:, :], axis=_AX.X, op=_ALU.add
                )
                nc.vector.tensor_reduce(
                    pr_all[:, rb : rb + 1], pr_cols[:, :], axis=_AX.X, op=_ALU.add
                )
                nc.vector.tensor_reduce(
                    ts_all[:, rb : rb + 1], ts_cols[:, :], axis=_AX.X, op=_ALU.add
                )

            lse = fin.tile([P, rb_count], _f32)
            nc.scalar.activation(lse[:, :], se_all[:, :], _AF.Ln)
            tl = fin.tile([P, rb_count], _f32)
            nc.vector.tensor_mul(tl[:, :], ts_all[:, :], lse[:, :])
            ce_rb = fin.tile([P, rb_count], _f32)
            nc.vector.tensor_sub(ce_rb[:, :], tl[:, :], pr_all[:, :])
            ce1 = fin.tile([P, 1], _f32)
            nc.vector.tensor_reduce(ce1[:, :], ce_rb[:, :], axis=_AX.X, op=_ALU.add)
            nc.sync.dma_start(out[:, :], ce1[:, :])

    nc.finalize()
    return nc


_cached_nc = None
_cached_runner = None


def _get_program():
    global _cached_nc
    if _cached_nc is None:
        _cached_nc = build_program()
    return _cached_nc


def _get_runner():
    """Build the sharded PJRT callable once (mirrors
    bass2jax.run_bass_via_pjrt, but reusable across kernel() calls)."""
    global _cached_runner
    if _cached_runner is not None:
        return _cached_runner

    import jax
    from concourse import bass2jax
    from jax.sharding import Mesh, PartitionSpec, NamedSharding
    from jax.experimental.shard_map import shard_map

    nc = _get_program()
    bass2jax.install_neuronx_cc_hook()

    partition_name = nc.partition_id_tensor.name if nc.partition_id_tensor else None
    in_names, out_names, out_avals = [], [], []
    for alloc in nc.m.functions[0].allocations:
        if not isinstance(alloc, mybir.MemoryLocationSet):
            continue
        name = alloc.memorylocations[0].name
        if alloc.kind == "ExternalInput":
            if name != partition_name:
                in_names.append(name)
        elif alloc.kind == "ExternalOutput":
            out_names.append(name)
            out_avals.append(
                jax.core.ShapedArray(
                    tuple(alloc.tensor_shape), mybir.dt.np(alloc.dtype)
                )
            )
    all_in_names = list(in_names) + list(out_names)
    if partition_name is not None:
        all_in_names.append(partition_name)
    n_params = len(in_names)

    def _body(*args):
        operands = list(args)
        if partition_name is not None:
            operands.append(bass2jax.partition_id_tensor())
        outs = bass2jax._bass_exec_p.bind(
            *operands,
            out_avals=tuple(out_avals),
            in_names=tuple(all_in_names),
            out_names=tuple(out_names),
            lowering_input_output_aliases=(),
            sim_require_finite=True,
            sim_require_nnan=True,
            nc=nc,
        )
        return tuple(outs)

    devices = jax.devices()[:N_CORES]
    mesh = Mesh(np.asarray(devices), ("core",))
    spec = PartitionSpec("core")
    fn = jax.jit(
        shard_map(
            _body,
            mesh=mesh,
            in_specs=(spec,) * (n_params + len(out_names)),
            out_specs=(spec,) * len(out_names),
            check_rep=False,
        ),
        keep_unused=True,
    )
    sharding = NamedSharding(mesh, spec)
    out_shapes = [a.shape for a in out_avals]
    _cached_runner = (fn, in_names, out_names, out_shapes, sharding, mesh, spec)
    return _cached_runner


def kernel(prediction: np.ndarray, target: np.ndarray) -> np.ndarray:
    import jax

    prediction = np.asarray(prediction, dtype=np.float32)
    target = np.asarray(target, dtype=np.float32)
    assert prediction.shape == (N, C) and target.shape == (N, C)

    fn, in_names, out_names, out_shapes, sharding, mesh, spec = _get_runner()
    by_name = {"prediction": prediction, "target": target}
    # async puts: both tensors may overlap in flight; fn blocks on arrival
    dev_in = [jax.device_put(by_name[n], sharding) for n in in_names]
    dev_zero = [
        jax.device_put(np.zeros((N_CORES * s[0], *s[1:]), np.float32), sharding)
        for s in out_shapes
    ]

    out = None
    for attempt in range(3):
        try:
            out = fn(*dev_in, *dev_zero)
            out = [np.asarray(o) for o in out]
            break
        except Exception:
            # A previously wedged device surfaces as an unrecoverable-exec
            # error on the first attempt and is reset by the runtime; retry.
            if attempt == 2:
                raise
    ce_part = out[out_names.index(OUT_NAME)]  # [N_CORES*P, 1]
    total = float(ce_part.astype(np.float64).sum())
    return np.asarray(total / N, dtype=np.float32)
